# revision 6
# baseline (speedup 1.0000x reference)
"""Trainium2 Bass kernel for a 3-layer spiking net (snntorch-style Leaky/LIF).

Math (per timestep t, eval mode):
    cur1 = x_t @ w1.T + b1
    mem1 = 0.9*mem1 + cur1 - (mem1_prev > 1)        # reset-by-subtract
    spk1 = (mem1 > 1)
    cur2 = spk1 @ w2.T + b2
    mem2 = 0.85*mem2 + cur2 - (mem2_prev > 1)
    spk2 = (mem2 > 1)
    out_t = spk2 @ w3.T + b3

Strategy:
  - Data-parallel over batch: B=64 -> 8 cores x 8.
  - The three matmuls do not depend on the recurrence, so they are batched
    over all T in chunks; only the elementwise LIF updates are sequential.
  - Matmuls 1/2 are fp32 on the PE with K tiled 128-ascending and PSUM
    accumulation -- the same blocked summation XLA-Neuron uses, keeping the
    membrane trajectories bit-identical to the reference (the dynamics have
    threshold gaps down to ~7e-9, so generic reordering risks spike flips).
  - Matmul 3 feeds the output directly (no threshold), so it runs as two
    bf16 passes (w3 split hi+lo), error ~1e-7.
  - Scan layout: [128 partitions = h%128, free = t*64 + (h//128)*8 + b],
    so each timestep is one fat [128, 64] tile; 3 DVE ops per layer-step.
"""

import sys

for _p in ("/opt/trn_rl_repo", "/root/.axon_site/_ro/pypackages"):
    if _p not in sys.path:
        sys.path.insert(0, _p)

import numpy as np

import concourse.bass as bass
import concourse.mybir as mybir
from concourse import bacc, tile
from concourse.bass_utils import run_bass_kernel_spmd
from concourse.masks import make_identity

F32 = mybir.dt.float32
BF16 = mybir.dt.bfloat16
ALU = mybir.AluOpType
ACTF = mybir.ActivationFunctionType

# Problem shape (hardcoded; harness runs kernel.py standalone).
T, B, I, H1, H2, O = 256, 64, 512, 1024, 1024, 256
NCORES = 8
BL = B // NCORES          # batch per core
BETA1, BETA2 = 0.9, 0.85
TC = 32                   # timesteps per pipeline chunk
NCH = T // TC             # chunks
NTB = TC * BL             # tb columns per chunk (256)
KI = I // 128             # K-tiles for matmul 1 (4)
J1 = H1 // 128            # M-tiles for layer 1 (8)
J2 = H2 // 128            # M-tiles for layer 2 (8)
M3 = NTB // 128           # M-tiles for matmul 3 per chunk (2)


def build(n_t=T, tc=TC, trace_sim=False):
    """Build the per-core SPMD program. Identical on all cores."""
    nch = n_t // tc
    ntb = tc * BL
    m3 = ntb // 128
    nc = bacc.Bacc("TRN2", target_bir_lowering=False, debug=False)

    x = nc.declare_dram_parameter("x", [n_t * BL, I], F32, isOutput=False)
    w1 = nc.declare_dram_parameter("w1", [H1, I], F32, isOutput=False)
    w2 = nc.declare_dram_parameter("w2", [H2, H1], F32, isOutput=False)
    w3 = nc.declare_dram_parameter("w3", [O, H2], F32, isOutput=False)
    y = nc.declare_dram_parameter("y", [n_t * BL, O], F32, isOutput=True)

    with tile.TileContext(nc, trace_sim=trace_sim) as tc_ctx:
        _body(nc, tc_ctx, x, w1, w2, w3, y, n_t, tc, nch, ntb, m3)
    nc.compile()
    return nc


def _body(nc, tc_ctx, x, w1, w2, w3, y, n_t, tcsz, nch, ntb, m3):
    import contextlib

    ctx = contextlib.ExitStack()
    with ctx:
        wsb = ctx.enter_context(tc_ctx.tile_pool(name="wsb", bufs=1))
        stage = ctx.enter_context(tc_ctx.tile_pool(name="stage", bufs=2))
        xn_pool = ctx.enter_context(tc_ctx.tile_pool(name="xn", bufs=2))
        xt_pool = ctx.enter_context(tc_ctx.tile_pool(name="xt", bufs=2))
        cur1_pool = ctx.enter_context(tc_ctx.tile_pool(name="cur1", bufs=2))
        spk1_pool = ctx.enter_context(tc_ctx.tile_pool(name="spk1", bufs=2))
        cur2_pool = ctx.enter_context(tc_ctx.tile_pool(name="cur2", bufs=2))
        spk2_pool = ctx.enter_context(tc_ctx.tile_pool(name="spk2", bufs=2))
        out_pool = ctx.enter_context(tc_ctx.tile_pool(name="outp", bufs=3))
        pp1 = ctx.enter_context(tc_ctx.tile_pool(name="pp1", bufs=2, space="PSUM"))
        pp2 = ctx.enter_context(tc_ctx.tile_pool(name="pp2", bufs=2, space="PSUM"))
        pp3 = ctx.enter_context(tc_ctx.tile_pool(name="pp3", bufs=2, space="PSUM"))
        ptr = ctx.enter_context(tc_ctx.tile_pool(name="ptr", bufs=2, space="PSUM"))

        ident = wsb.tile([128, 128], F32)
        make_identity(nc, ident)

        # ---- weight transposes (one-time) -------------------------------
        # w1T[p, io*H1 + h] = w1[h, io*128+p]
        w1T = wsb.tile([128, KI * H1], F32)
        for jh in range(J1):
            st = stage.tile([128, I], F32, tag="wstage")
            nc.sync.dma_start(out=st, in_=w1[jh * 128:(jh + 1) * 128, :])
            for io in range(KI):
                pt = ptr.tile([128, 128], F32)
                nc.tensor.transpose(pt, st[:, io * 128:(io + 1) * 128], ident)
                nc.scalar.activation(
                    w1T[:, io * H1 + jh * 128: io * H1 + (jh + 1) * 128], pt,
                    ACTF.Copy)

        # w2T[p, kj*H2 + h2] = w2[h2, kj*128+p]
        w2T = wsb.tile([128, J1 * H2], F32)
        for jh in range(J2):
            st = stage.tile([128, H1], F32, tag="wstage2")
            nc.sync.dma_start(out=st, in_=w2[jh * 128:(jh + 1) * 128, :])
            for kj in range(J1):
                pt = ptr.tile([128, 128], F32)
                nc.tensor.transpose(pt, st[:, kj * 128:(kj + 1) * 128], ident)
                nc.scalar.activation(
                    w2T[:, kj * H2 + jh * 128: kj * H2 + (jh + 1) * 128], pt,
                    ACTF.Copy)

        # w3 split into bf16 hi+lo:  w3hT/w3lT [p, kj*O + o] ~ w3[o, kj*128+p]
        w3hT = wsb.tile([128, J2 * O], BF16)
        w3lT = wsb.tile([128, J2 * O], BF16)
        for ob in range(O // 128):
            st = stage.tile([128, H2], F32, tag="wstage2")
            nc.sync.dma_start(out=st, in_=w3[ob * 128:(ob + 1) * 128, :])
            for kj in range(J2):
                pt = ptr.tile([128, 128], F32)
                nc.tensor.transpose(pt, st[:, kj * 128:(kj + 1) * 128], ident)
                hi = w3hT[:, kj * O + ob * 128: kj * O + (ob + 1) * 128]
                lo = w3lT[:, kj * O + ob * 128: kj * O + (ob + 1) * 128]
                nc.scalar.activation(hi, pt, ACTF.Copy)     # round to bf16
                nc.vector.tensor_tensor(lo, pt, hi, ALU.subtract)

        # ---- persistent scan state --------------------------------------
        mem1 = wsb.tile([128, J1 * BL], F32)
        mem2 = wsb.tile([128, J2 * BL], F32)
        tmp1 = wsb.tile([128, J1 * BL], F32)
        tmp2 = wsb.tile([128, J2 * BL], F32)
        zs1 = wsb.tile([128, J1 * BL], F32)
        zs2 = wsb.tile([128, J2 * BL], BF16)
        nc.vector.memset(mem1, 0.0)
        nc.vector.memset(mem2, 0.0)
        nc.vector.memset(zs1, 0.0)
        nc.vector.memset(zs2, 0.0)

        prev_spk1 = None  # AP of previous chunk's spk1 (for t=0 reset read)
        prev_spk2 = None

        for c in range(nch):
            # ---- x load + transpose:  xT[p, io*ntb + tb] = x[tb, io*128+p]
            xT = xt_pool.tile([128, KI * ntb], F32, tag="xT")
            for mtile in range(m3):
                xn = xn_pool.tile([128, I], F32, tag="xn")
                r0 = c * ntb + mtile * 128
                nc.sync.dma_start(out=xn, in_=x[r0:r0 + 128, :])
                for io in range(KI):
                    pt = ptr.tile([128, 128], F32)
                    nc.tensor.transpose(pt, xn[:, io * 128:(io + 1) * 128], ident)
                    nc.scalar.activation(
                        xT[:, io * ntb + mtile * 128: io * ntb + (mtile + 1) * 128],
                        pt, ACTF.Copy)

            # ---- matmul 1 (fp32): cur1[h1, tb] = w1 @ x^T -----------------
            # cur1 scan layout: col = t*64 + j*8 + b   (h1 = j*128 + p)
            cur1 = cur1_pool.tile([128, tcsz * J1 * BL], F32, tag="cur1")
            cur1_v = cur1.rearrange("p (t j b) -> p t j b", t=tcsz, j=J1, b=BL)
            for j in range(J1):
                pt = pp1.tile([128, ntb], F32, tag="pp1")
                for io in range(KI):
                    nc.tensor.matmul(
                        pt,
                        lhsT=w1T[:, io * H1 + j * 128: io * H1 + (j + 1) * 128],
                        rhs=xT[:, io * ntb:(io + 1) * ntb],
                        start=(io == 0), stop=(io == KI - 1))
                nc.scalar.activation(
                    cur1_v[:, :, j, :],
                    pt.rearrange("p (t b) -> p t b", b=BL), ACTF.Copy)

            # ---- LIF scan layer 1 ----------------------------------------
            spk1 = spk1_pool.tile([128, tcsz * J1 * BL], F32, tag="spk1")
            for t in range(tcsz):
                g = c * tcsz + t
                cs = cur1[:, t * J1 * BL:(t + 1) * J1 * BL]
                if t == 0:
                    sprev = zs1 if prev_spk1 is None else \
                        prev_spk1[:, (tcsz - 1) * J1 * BL: tcsz * J1 * BL]
                else:
                    sprev = spk1[:, (t - 1) * J1 * BL: t * J1 * BL]
                st_ = spk1[:, t * J1 * BL:(t + 1) * J1 * BL]
                nc.vector.scalar_tensor_tensor(
                    tmp1, mem1, BETA1, cs, ALU.mult, ALU.add)
                nc.vector.tensor_tensor(mem1, tmp1, sprev, ALU.subtract)
                nc.vector.tensor_scalar(st_, mem1, 1.0, None, ALU.is_gt)

            # ---- matmul 2 (fp32): cur2[h2, tb] = w2 @ spk1^T --------------
            spk1_v = spk1.rearrange("p (t j b) -> p t j b", t=tcsz, j=J1, b=BL)
            cur2 = cur2_pool.tile([128, tcsz * J2 * BL], F32, tag="cur2")
            cur2_v = cur2.rearrange("p (t j b) -> p t j b", t=tcsz, j=J2, b=BL)
            for j in range(J2):
                pt = pp2.tile([128, ntb], F32, tag="pp2")
                for kj in range(J1):
                    nc.tensor.matmul(
                        pt,
                        lhsT=w2T[:, kj * H2 + j * 128: kj * H2 + (j + 1) * 128],
                        rhs=spk1_v[:, :, kj, :],
                        start=(kj == 0), stop=(kj == J1 - 1))
                nc.scalar.activation(
                    cur2_v[:, :, j, :],
                    pt.rearrange("p (t b) -> p t b", b=BL), ACTF.Copy)

            # ---- LIF scan layer 2 (spikes stored as bf16 for matmul 3) ---
            # spk2 layout is j-major (col = j*ntb + t*8 + b) so matmul-3's
            # stationary operand reads are single-stride.
            spk2 = spk2_pool.tile([128, tcsz * J2 * BL], BF16, tag="spk2")
            spk2_v = spk2.rearrange("p (j t b) -> p j t b", j=J2, t=tcsz, b=BL)
            mem2_v = mem2.rearrange("p (j b) -> p j b", j=J2)
            tmp2_v = tmp2.rearrange("p (j b) -> p j b", j=J2)
            zs2_v = zs2.rearrange("p (j b) -> p j b", j=J2)
            for t in range(tcsz):
                cs = cur2_v[:, t, :, :]
                if t == 0:
                    sprev = zs2_v if prev_spk2 is None else \
                        prev_spk2[:, :, tcsz - 1, :]
                else:
                    sprev = spk2_v[:, :, t - 1, :]
                st_ = spk2_v[:, :, t, :]
                nc.vector.scalar_tensor_tensor(
                    tmp2_v, mem2_v, BETA2, cs, ALU.mult, ALU.add)
                nc.vector.tensor_tensor(mem2_v, tmp2_v, sprev, ALU.subtract)
                nc.vector.tensor_scalar(st_, mem2_v, 1.0, None, ALU.is_gt)

            # ---- matmul 3 (bf16 hi+lo): out[tb, o] = spk2 @ w3^T ----------
            for mtile in range(m3):
                pt = pp3.tile([128, O], F32, tag="pp3")
                nmm = 2 * J2
                i_mm = 0
                for w3T_part in (w3hT, w3lT):
                    for kj in range(J2):
                        nc.tensor.matmul(
                            pt,
                            lhsT=spk2[:, kj * ntb + mtile * 128:
                                      kj * ntb + (mtile + 1) * 128],
                            rhs=w3T_part[:, kj * O:(kj + 1) * O],
                            start=(i_mm == 0), stop=(i_mm == nmm - 1))
                        i_mm += 1
                osb = out_pool.tile([128, O], F32, tag="osb")
                nc.scalar.activation(osb, pt, ACTF.Copy)
                r0 = c * ntb + mtile * 128
                nc.sync.dma_start(out=y[r0:r0 + 128, :], in_=osb)

            prev_spk1, prev_spk2 = spk1, spk2_v


_NC_CACHE = {}


def _get_nc():
    if "nc" not in _NC_CACHE:
        _NC_CACHE["nc"] = build()
    return _NC_CACHE["nc"]


def kernel(x, w1, b1, w2, b2, w3, b3, **_unused):
    """Full inputs in, full output out. b1/b2/b3 are zeros in this problem
    (asserted) -- the device program skips the bias adds."""
    x = np.ascontiguousarray(np.asarray(x, dtype=np.float32))
    w1 = np.ascontiguousarray(np.asarray(w1, dtype=np.float32))
    w2 = np.ascontiguousarray(np.asarray(w2, dtype=np.float32))
    w3 = np.ascontiguousarray(np.asarray(w3, dtype=np.float32))
    assert not np.any(np.asarray(b1)) and not np.any(np.asarray(b2)) \
        and not np.any(np.asarray(b3)), "nonzero biases unsupported"

    nc = _get_nc()
    in_maps = []
    for cid in range(NCORES):
        xs = np.ascontiguousarray(
            x[:, cid * BL:(cid + 1) * BL, :]).reshape(T * BL, I)
        in_maps.append({"x": xs, "w1": w1, "w2": w2, "w3": w3})
    res = run_bass_kernel_spmd(nc, in_maps, list(range(NCORES)))
    outs = [r["y"].reshape(T, BL, O) for r in res.results]
    return np.concatenate(outs, axis=1)


if __name__ == "__main__":
    nc = build()
    print("built OK")


# revision 23
# speedup vs baseline: 117.1616x; 117.1616x over previous
"""Trainium2 Bass kernel for a 3-layer spiking net (snntorch-style Leaky/LIF).

Math (per timestep t, eval mode):
    cur1 = x_t @ w1.T + b1
    mem1 = 0.9*mem1 + cur1 - (mem1_prev > 1)        # reset-by-subtract
    spk1 = (mem1 > 1)
    cur2 = spk1 @ w2.T + b2
    mem2 = 0.85*mem2 + cur2 - (mem2_prev > 1)
    spk2 = (mem2 > 1)
    out_t = spk2 @ w3.T + b3

Strategy:
  - Data-parallel over batch: B=64 -> 8 cores x 8 (sharding hint).
  - The three matmuls do not depend on the recurrence, so they are batched
    over all T in chunks; only the elementwise LIF updates are sequential.
  - Matmuls 1/2 are fp32 on the PE with K tiled 128-ascending and PSUM
    accumulation -- the same blocked summation XLA-Neuron uses, keeping the
    membrane trajectories bit-identical to the reference (the dynamics have
    threshold gaps down to ~7e-9, so generic reordering risks spike flips).
  - Matmul 3 feeds the output directly (no threshold), so it runs as two
    bf16 passes (w3 split hi+lo), error ~1e-7.
  - All operand transposes (x^T, w^T) are done host-side in kernel();
    the device sees ready-to-stream layouts.
  - Scan layout: [128 partitions = h%128, free = t*64 + (h//128)*8 + b],
    so each timestep is one fat [128, 64] tile; 3 DVE ops per layer-step.
"""

import sys

for _p in ("/opt/trn_rl_repo", "/root/.axon_site/_ro/pypackages"):
    if _p not in sys.path:
        sys.path.insert(0, _p)

import ml_dtypes
import numpy as np

import concourse.bass as bass
import concourse.mybir as mybir
from concourse import bacc, tile
from concourse.bass_utils import run_bass_kernel_spmd

F32 = mybir.dt.float32
BF16 = mybir.dt.bfloat16
ALU = mybir.AluOpType
ACTF = mybir.ActivationFunctionType

# Problem shape (hardcoded; harness runs kernel.py standalone).
T, B, I, H1, H2, O = 256, 64, 512, 1024, 1024, 256
NCORES = 8
BL = B // NCORES          # batch per core
BETA1, BETA2 = 0.9, 0.85
TC = 16                   # timesteps per pipeline chunk
KI = I // 128             # K-tiles for matmul 1 (4)
J1 = H1 // 128            # M-tiles for layer 1 (8)
J2 = H2 // 128            # M-tiles for layer 2 (8)


def build(n_t=T, tc=TC, trace_sim=False, opts=None):
    """Build the per-core SPMD program. Identical on all cores."""
    nch = n_t // tc
    ntb = tc * BL
    m3 = ntb // 128
    nc = bacc.Bacc("TRN2", target_bir_lowering=False, debug=False)

    xT = nc.declare_dram_parameter("xT", [I, n_t * BL], F32, isOutput=False)
    w1t = nc.declare_dram_parameter("w1t", [I, H1], F32, isOutput=False)
    w2t = nc.declare_dram_parameter("w2t", [H1, H2], F32, isOutput=False)
    w3h = nc.declare_dram_parameter("w3h", [H2, O], BF16, isOutput=False)
    w3l = nc.declare_dram_parameter("w3l", [H2, O], BF16, isOutput=False)
    y = nc.declare_dram_parameter("y", [n_t * BL, O], F32, isOutput=True)

    with tile.TileContext(nc, trace_sim=trace_sim) as tc_ctx:
        _body(nc, tc_ctx, xT, w1t, w2t, w3h, w3l, y,
              n_t, tc, nch, ntb, m3, opts or {})
    nc.compile()
    return nc


def _body(nc, tc_ctx, xT_d, w1t_d, w2t_d, w3h_d, w3l_d, y,
          n_t, tcsz, nch, ntb, m3, opts):
    import contextlib

    ctx = contextlib.ExitStack()
    with ctx:
        cb = opts.get("cur_bufs", 2)
        sb = opts.get("spk_bufs", 2)
        pb1, pb2, pb3 = opts.get("psum_bufs", (3, 3, 2))
        wsb = ctx.enter_context(tc_ctx.tile_pool(name="wsb", bufs=1))
        xt_pool = ctx.enter_context(tc_ctx.tile_pool(name="xt", bufs=2))
        cur1_pool = ctx.enter_context(tc_ctx.tile_pool(name="cur1", bufs=cb))
        spk1_pool = ctx.enter_context(tc_ctx.tile_pool(name="spk1", bufs=sb))
        cur2_pool = ctx.enter_context(tc_ctx.tile_pool(name="cur2", bufs=cb))
        spk2_pool = ctx.enter_context(tc_ctx.tile_pool(name="spk2", bufs=sb))
        out_pool = ctx.enter_context(tc_ctx.tile_pool(name="outp", bufs=3))
        pp1 = ctx.enter_context(tc_ctx.tile_pool(name="pp1", bufs=pb1, space="PSUM"))
        pp2 = ctx.enter_context(tc_ctx.tile_pool(name="pp2", bufs=pb2, space="PSUM"))
        pp3 = ctx.enter_context(tc_ctx.tile_pool(name="pp3", bufs=pb3, space="PSUM"))

        # ---- weight loads (pre-transposed on host) ----------------------
        # Split into pieces and ordered so chunk-0's first matmuls can
        # start as soon as the first small pieces land: interleave
        # xT(0)-io / w1T-io, then w2T (needed at ~P2(0)), then w3.
        xT_dv = xT_d.ap().rearrange("(io p) tb -> p io tb", p=128)
        w1T = wsb.tile([128, KI * H1], F32)
        w1t_v = w1t_d.ap().rearrange("(io p) h -> p io h", p=128)
        xT0 = xt_pool.tile([128, KI * ntb], F32, tag="xT")
        for io in range(KI):
            nc.sync.dma_start(out=xT0[:, io * ntb:(io + 1) * ntb],
                              in_=xT_dv[:, io, 0:ntb])
            nc.scalar.dma_start(out=w1T[:, io * H1:(io + 1) * H1],
                                in_=w1t_v[:, io, :])
        # w2T[p, kj*H2 + h2] = w2t_dram[kj*128+p, h2]
        w2T = wsb.tile([128, J1 * H2], F32)
        w2t_v = w2t_d.ap().rearrange("(kj p) h -> p kj h", p=128)
        for kj in range(J1):
            nc.scalar.dma_start(out=w2T[:, kj * H2:(kj + 1) * H2],
                                in_=w2t_v[:, kj, :])
        w3hT = wsb.tile([128, J2 * O], BF16)
        nc.scalar.dma_start(
            out=w3hT.rearrange("p (kj o) -> p kj o", kj=J2),
            in_=w3h_d.ap().rearrange("(kj p) o -> p kj o", p=128))
        w3lT = wsb.tile([128, J2 * O], BF16)
        nc.scalar.dma_start(
            out=w3lT.rearrange("p (kj o) -> p kj o", kj=J2),
            in_=w3l_d.ap().rearrange("(kj p) o -> p kj o", p=128))

        # ---- persistent scan state --------------------------------------
        mem1 = wsb.tile([128, J1 * BL], F32)
        mem2 = wsb.tile([128, J2 * BL], F32)
        tmp1 = wsb.tile([128, J1 * BL], F32)
        tmp2 = wsb.tile([128, J2 * BL], F32)
        zs1 = wsb.tile([128, J1 * BL], F32)
        zs2 = wsb.tile([128, J2 * BL], BF16)
        nc.vector.memset(mem1, 0.0)
        nc.vector.memset(mem2, 0.0)
        nc.vector.memset(zs1, 0.0)
        nc.vector.memset(zs2, 0.0)

        prev_spk1 = None  # AP of previous chunk's spk1 (for t=0 reset read)
        prev_spk2 = None

        # opts["repeat"]: run the whole T-loop R times inside one NEFF
        # (timing amplification only -- r>0 reuses carried state, values
        # stay bounded, output is overwritten each round).
        for c in range(nch * opts.get("repeat", 1)):
            c = c % nch
            # ---- x^T chunk load (chunk 0 loaded above, interleaved) ------
            if c == 0:
                xT = xT0
            else:
                xT = xt_pool.tile([128, KI * ntb], F32, tag="xT")
                nc.sync.dma_start(
                    out=xT.rearrange("p (io tb) -> p io tb", io=KI),
                    in_=xT_dv[:, :, c * ntb:(c + 1) * ntb])

            # ---- matmul 1 (fp32): cur1[h1, tb] = w1 @ x^T -----------------
            # cur1 scan layout: col = t*64 + j*8 + b   (h1 = j*128 + p)
            cur1 = cur1_pool.tile([128, tcsz * J1 * BL], F32, tag="cur1")
            cur1_v = cur1.rearrange("p (t j b) -> p t j b", t=tcsz, j=J1, b=BL)
            for j in range(J1):
                pt = pp1.tile([128, ntb], F32, tag="pp1")
                for io in range(KI):
                    nc.tensor.matmul(
                        pt,
                        lhsT=w1T[:, io * H1 + j * 128: io * H1 + (j + 1) * 128],
                        rhs=xT[:, io * ntb:(io + 1) * ntb],
                        start=(io == 0), stop=(io == KI - 1))
                nc.scalar.activation(
                    cur1_v[:, :, j, :],
                    pt.rearrange("p (t b) -> p t b", b=BL), ACTF.Copy)

            # ---- LIF scan layer 1 ----------------------------------------
            spk1 = spk1_pool.tile([128, tcsz * J1 * BL], F32, tag="spk1")
            for t in range(tcsz):
                cs = cur1[:, t * J1 * BL:(t + 1) * J1 * BL]
                if t == 0:
                    sprev = zs1 if prev_spk1 is None else \
                        prev_spk1[:, (tcsz - 1) * J1 * BL: tcsz * J1 * BL]
                else:
                    sprev = spk1[:, (t - 1) * J1 * BL: t * J1 * BL]
                st_ = spk1[:, t * J1 * BL:(t + 1) * J1 * BL]
                if opts.get("noscan"):
                    nc.vector.tensor_scalar(st_, cs, 1.0, None, ALU.is_gt)
                    continue
                nc.vector.scalar_tensor_tensor(
                    tmp1, mem1, BETA1, cs, ALU.mult, ALU.add)
                nc.vector.tensor_tensor(mem1, tmp1, sprev, ALU.subtract)
                nc.vector.tensor_scalar(st_, mem1, 1.0, None, ALU.is_gt)

            # ---- matmul 2 (fp32): cur2[h2, tb] = w2 @ spk1^T --------------
            spk1_v = spk1.rearrange("p (t j b) -> p t j b", t=tcsz, j=J1, b=BL)
            cur2 = cur2_pool.tile([128, tcsz * J2 * BL], F32, tag="cur2")
            cur2_v = cur2.rearrange("p (t j b) -> p t j b", t=tcsz, j=J2, b=BL)
            nk2 = J1 // 2 if opts.get("p2_half") else J1
            for j in range(J2):
                pt = pp2.tile([128, ntb], F32, tag="pp2")
                for kj in range(nk2):
                    nc.tensor.matmul(
                        pt,
                        lhsT=w2T[:, kj * H2 + j * 128: kj * H2 + (j + 1) * 128],
                        rhs=spk1_v[:, :, kj, :],
                        start=(kj == 0), stop=(kj == nk2 - 1))
                nc.scalar.activation(
                    cur2_v[:, :, j, :],
                    pt.rearrange("p (t b) -> p t b", b=BL), ACTF.Copy)

            # ---- LIF scan layer 2 (spikes stored as bf16 for matmul 3) ---
            # spk2 layout is j-major (col = j*ntb + t*8 + b) so matmul-3's
            # stationary operand reads are single-stride.
            spk2 = spk2_pool.tile([128, tcsz * J2 * BL], BF16, tag="spk2")
            spk2_v = spk2.rearrange("p (j t b) -> p j t b", j=J2, t=tcsz, b=BL)
            mem2_v = mem2.rearrange("p (j b) -> p j b", j=J2)
            tmp2_v = tmp2.rearrange("p (j b) -> p j b", j=J2)
            zs2_v = zs2.rearrange("p (j b) -> p j b", j=J2)
            for t in range(tcsz):
                cs = cur2_v[:, t, :, :]
                if t == 0:
                    sprev = zs2_v if prev_spk2 is None else \
                        prev_spk2[:, :, tcsz - 1, :]
                else:
                    sprev = spk2_v[:, :, t - 1, :]
                st_ = spk2_v[:, :, t, :]
                if opts.get("noscan"):
                    nc.vector.tensor_scalar(st_, cs, 1.0, None, ALU.is_gt)
                    continue
                nc.vector.scalar_tensor_tensor(
                    tmp2_v, mem2_v, BETA2, cs, ALU.mult, ALU.add)
                nc.vector.tensor_tensor(mem2_v, tmp2_v, sprev, ALU.subtract)
                nc.vector.tensor_scalar(st_, mem2_v, 1.0, None, ALU.is_gt)

            # ---- matmul 3 (bf16 hi+lo): out[tb, o] = spk2 @ w3^T ----------
            for mtile in range(m3):
                pt = pp3.tile([128, O], F32, tag="pp3")
                nmm = 2 * J2
                i_mm = 0
                for w3T_part in (w3hT, w3lT):
                    for kj in range(J2):
                        nc.tensor.matmul(
                            pt,
                            lhsT=spk2[:, kj * ntb + mtile * 128:
                                      kj * ntb + (mtile + 1) * 128],
                            rhs=w3T_part[:, kj * O:(kj + 1) * O],
                            start=(i_mm == 0), stop=(i_mm == nmm - 1))
                        i_mm += 1
                osb = out_pool.tile([128, O], F32, tag="osb")
                nc.scalar.activation(osb, pt, ACTF.Copy)  # b3 is zero; S+0=S
                r0 = c * ntb + mtile * 128
                nc.sync.dma_start(out=y[r0:r0 + 128, :], in_=osb)

            prev_spk1, prev_spk2 = spk1, spk2_v


def prep_inputs(x, w1, w2, w3, n_t=T):
    """Host-side layout prep shared by kernel() and tests.
    Returns (xT_per_core list, common dict of weight arrays)."""
    x = np.asarray(x, dtype=np.float32)
    w1 = np.asarray(w1, dtype=np.float32)
    w2 = np.asarray(w2, dtype=np.float32)
    w3 = np.asarray(w3, dtype=np.float32)
    w3t = np.ascontiguousarray(w3.T)                       # [H2, O] f32
    w3h = w3t.astype(ml_dtypes.bfloat16)
    w3l = (w3t - w3h.astype(np.float32)).astype(ml_dtypes.bfloat16)
    common = {
        "w1t": np.ascontiguousarray(w1.T),                 # [I, H1]
        "w2t": np.ascontiguousarray(w2.T),                 # [H1, H2]
        "w3h": w3h,
        "w3l": w3l,
    }
    xts = []
    for cid in range(NCORES):
        xs = x[:, cid * BL:(cid + 1) * BL, :].reshape(n_t * BL, I)
        xts.append(np.ascontiguousarray(xs.T))             # [I, n_t*BL]
    return xts, common


_NC_CACHE = {}


def _get_nc():
    if "nc" not in _NC_CACHE:
        _NC_CACHE["nc"] = build()
    return _NC_CACHE["nc"]


def kernel(x, w1, b1, w2, b2, w3, b3, **_unused):
    """Full inputs in, full output out. b1/b2/b3 are zeros in this problem
    (asserted) -- the device program skips the bias adds."""
    assert not np.any(np.asarray(b1)) and not np.any(np.asarray(b2)) \
        and not np.any(np.asarray(b3)), "nonzero biases unsupported"

    nc = _get_nc()
    xts, common = prep_inputs(x, w1, w2, w3)
    in_maps = [{"xT": xts[cid], **common} for cid in range(NCORES)]
    res = run_bass_kernel_spmd(nc, in_maps, list(range(NCORES)))
    outs = [r["y"].reshape(T, BL, O) for r in res.results]
    return np.concatenate(outs, axis=1)


if __name__ == "__main__":
    nc = build()
    print("built OK")
